# revision 3
# baseline (speedup 1.0000x reference)
"""Legendre polynomials P_0..P_11 (Bonnet recurrence) on 8 TRN2 NeuronCores.

Input:  x float32 [16777216]  (angle cosines in [-1, 1])
Output: float32 [16777216, 12],  out[i, j] = P_j(x[i])

v3: balanced two-engine schedule (v2 post-mortem: GPSIMD tensor_scalar
contends with DVE fp16 tensor_tensor for the shared SBUF port pair --
TT avg inflated 1309 -> 2913 ns -- so GPSIMD is unusable while the DVE
chain runs; v2 hit 242us vs v1's 192us).

  * DVE (19.8us/tile): the 8 TTs (mul3, mul5, 3 pair-chain steps of
    [P,2M] mul+sub) plus the 4 cheapest fp16 4x tensor_scalar ops
    (r, pl4, v5, s9).
  * ACT (17.9us/tile): y=x^2, q4, q5, pl2, s4..s8 (9 ACTIVATE).
  * GPSIMD: nothing.
  * plane2 is stored AS the k=4 chain subtrahend C2*(1.5y-0.5); r and
    s9 are decoded from it by fp16 TS (affine of an affine).
  * x16 comes from the host (pure dtype cast, like the output decode);
    DMA 52.5 MB/core stays under the compute walls.
  * planes live in ONE contiguous [P, 10*M] fp16 tile; 3 output
    dma_starts/tile ([0:4M) planes 2-5, [4M:8M) 6-9, [8M:10M) 10-11)
    keep the store stream smooth.

Host: P0/P1 fill + per-plane fp16 scale decode (pure format transform).
Host-simulated worst-case abs error of this pipeline: 6.0e-3 (gate 2e-2).
"""

import numpy as np

import concourse.bass as bass
import concourse.tile as tile
from concourse import bacc, mybir
from concourse.bass_utils import run_bass_kernel_spmd

N = 16777216
N_CORES = 8
S = N // N_CORES      # 2097152 elements per core
P = 128               # SBUF partitions
M = 2048              # free-dim elements per plane per tile
T = S // (P * M)      # 8 tiles per core
NORD = 12
NPLANES = 10          # device-computed orders 2..11

F32 = mybir.dt.float32
F16 = mybir.dt.float16


def _chain_coef():
    def a(k):
        return (2 * k + 1) / (k + 1)

    def b(k):
        return k / (k + 1)

    coef = {}
    for k in range(4, 10):
        A = a(k + 1) * a(k)
        B = -(b(k + 1) + a(k + 1) * b(k) / a(k - 1))
        G = a(k + 1) * b(k) * b(k - 1) / a(k - 1)
        coef[k] = (A, B, G)
    return coef


CHAIN = _chain_coef()
G4, G5, G6, G7, G8, G9 = (CHAIN[k][2] for k in range(4, 10))

# plane scales: plane_j = C_j * P_j  (host multiplies by 1/C_j)
C3, C5, C6 = 1.0, 1.0, 1.0
C7, C9 = C3 / G5, C5 / G7
C8, C10, C11 = 1.0 / G6, 1.0 / G8, C7 / G9
PHAT4 = C8 * G6            # plane4 = PHAT4 * P4  (the k=6 subtrahend)
C2 = G4 * C6               # plane2 = C2*(1.5y-0.5) (the k=4 subtrahend)

FK = {4: C6 / PHAT4, 5: C7 / C5, 6: C8 / C6, 7: C9 / C7,
      8: C10 / C8, 9: C11 / C9}

# host decode: P_j = HOST_SCALE[j] * plane_j  (pure scales, no shifts)
HOST_SCALE = {2: 1.0 / C2, 3: 1.0 / C3, 4: 1.0 / PHAT4, 5: 1.0 / C5,
              6: 1.0 / C6, 7: 1.0 / C7, 8: 1.0 / C8, 9: 1.0 / C9,
              10: 1.0 / C10, 11: 1.0 / C11}

_NC_CACHE = {}


def build_nc():
    if "nc" in _NC_CACHE:
        return _NC_CACHE["nc"]
    nc = bacc.Bacc("TRN2", target_bir_lowering=False, debug=False,
                   num_devices=N_CORES)
    x = nc.declare_dram_parameter("x", [T, P, M], F32, isOutput=False)
    xh = nc.declare_dram_parameter("x16", [T, P, M], F16, isOutput=False)
    out = nc.declare_dram_parameter("out", [T, P, NPLANES * M], F16,
                                    isOutput=True)

    ACT = mybir.ActivationFunctionType
    ALU = mybir.AluOpType

    with tile.TileContext(nc) as tc:
        with (
            tc.tile_pool(name="cbias", bufs=1) as cpool,
            tc.tile_pool(name="xin", bufs=3) as xpool,
            tc.tile_pool(name="x16", bufs=3) as hpool,
            tc.tile_pool(name="ysq", bufs=2) as ypool,
            tc.tile_pool(name="mega", bufs=2) as mpool,
            tc.tile_pool(name="schain", bufs=6) as spool,
            tc.tile_pool(name="aff", bufs=6) as affpool,
        ):
            b47 = cpool.tile([P, 1], F32)
            nc.vector.memset(b47[:], -3.0 / 7.0)
            b59 = cpool.tile([P, 1], F32)
            nc.vector.memset(b59[:], -5.0 / 9.0)

            xts = {}

            def load_x(t):
                xts[t] = (xpool.tile([P, M], F32, tag="xt", name=f"xt_{t}"),
                          hpool.tile([P, M], F16, tag="xh", name=f"xh_{t}"))
                nc.sync.dma_start(xts[t][0][:], x[t])
                nc.sync.dma_start(xts[t][1][:], xh[t])

            load_x(0)
            load_x(1)
            for t in range(T):
                if t + 2 < T:
                    load_x(t + 2)
                xt, x16 = xts.pop(t)

                mega = mpool.tile([P, NPLANES * M], F16, tag="mega",
                                  name=f"mega_{t}")
                pl = {j: mega[:, (j - 2) * M:(j - 1) * M]
                      for j in range(2, 12)}

                yt = ypool.tile([P, M], F32, tag="yt", name=f"yt_{t}")
                nc.scalar.activation(yt[:], xt[:], ACT.Square)

                # plane2 = C2*(1.5y - 0.5)  == the k=4 chain subtrahend (ACT)
                nc.scalar.activation(pl[2], yt[:], ACT.Copy,
                                     bias=-0.5 * C2, scale=1.5 * C2)

                # r = 2.5y - 1.5 recovered from plane2 (DVE 4x TS)
                r = affpool.tile([P, M], F16, tag="aff", name=f"r_{t}")
                nc.vector.tensor_scalar(r[:], pl[2], 5.0 / (3.0 * C2),
                                        -2.0 / 3.0, ALU.mult, ALU.add)
                # plane3 = x16 * r   (DVE TT)
                nc.vector.tensor_mul(pl[3], x16[:], r[:])

                # plane4 = PHAT4*(4.375 q4 - 3/7),  q4 = (y - 3/7)^2
                q4 = affpool.tile([P, M], F16, tag="aff", name=f"q4_{t}")
                nc.scalar.activation(q4[:], yt[:], ACT.Square, bias=b47[:])
                nc.vector.tensor_scalar(pl[4], q4[:], 4.375 * PHAT4,
                                        -(3.0 / 7.0) * PHAT4,
                                        ALU.mult, ALU.add)

                # plane5 = x16 * (7.875 q5 - 5/9),  q5 = (y - 5/9)^2
                q5 = affpool.tile([P, M], F16, tag="aff", name=f"q5_{t}")
                nc.scalar.activation(q5[:], yt[:], ACT.Square, bias=b59[:])
                v5 = affpool.tile([P, M], F16, tag="aff", name=f"v5_{t}")
                nc.vector.tensor_scalar(v5[:], q5[:], 7.875, -5.0 / 9.0,
                                        ALU.mult, ALU.add)
                nc.vector.tensor_mul(pl[5], x16[:], v5[:])

                # s-pair tiles: [s_k | s_{k+1}], s_k = (A_k y + B_k) * fk
                # s4..s8 on ACT from f32 y; s9 on DVE 4x TS from plane2.
                spairs = {}
                for k in (4, 6, 8):
                    sp = spool.tile([P, 2 * M], F16, tag="s",
                                    name=f"s{k}{k + 1}_{t}")
                    for kk, half in ((k, sp[:, 0:M]), (k + 1, sp[:, M:2 * M])):
                        A, B, _G = CHAIN[kk]
                        if kk == 9:
                            sc = A * FK[kk] / (1.5 * C2)
                            bi = A * FK[kk] / 3.0 + B * FK[kk]
                            nc.vector.tensor_scalar(half, pl[2], sc, bi,
                                                    ALU.mult, ALU.add)
                        else:
                            nc.scalar.activation(half, yt[:], ACT.Copy,
                                                 bias=B * FK[kk],
                                                 scale=A * FK[kk])
                    spairs[k] = sp

                # pair chain: planes (k+2,k+3) = s_pair .* (k,k+1) - (k-2,k-1)
                for k in (4, 6, 8):
                    src = mega[:, (k - 2) * M:k * M]
                    sub = mega[:, (k - 4) * M:(k - 2) * M]
                    dst = mega[:, k * M:(k + 2) * M]
                    nc.vector.tensor_mul(dst, spairs[k][:], src)
                    nc.vector.tensor_sub(dst, dst, sub)
                    if k == 4:
                        nc.sync.dma_start(out[t][:, 0:4 * M],
                                          mega[:, 0:4 * M])
                    elif k == 6:
                        nc.sync.dma_start(out[t][:, 4 * M:8 * M],
                                          mega[:, 4 * M:8 * M])
                    else:
                        nc.sync.dma_start(out[t][:, 8 * M:NPLANES * M],
                                          mega[:, 8 * M:NPLANES * M])
    nc.compile()
    _NC_CACHE["nc"] = nc
    return nc


def run_device(x_full, trace=False, **kw):
    nc = build_nc()
    x16_full = x_full.astype(np.float16)
    in_maps = [
        {"x": np.ascontiguousarray(x_full[c * S:(c + 1) * S].reshape(T, P, M)),
         "x16": np.ascontiguousarray(
             x16_full[c * S:(c + 1) * S].reshape(T, P, M))}
        for c in range(N_CORES)
    ]
    return run_bass_kernel_spmd(nc, in_maps, core_ids=list(range(N_CORES)),
                                trace=trace, **kw)


def kernel(x):
    x = np.asarray(x, dtype=np.float32)
    res = run_device(x)
    full = np.empty((N, NORD), np.float32)
    full[:, 0] = 1.0          # P0 == 1 (constant; no compute involved)
    full[:, 1] = x            # P1 == x (identity; no compute involved)
    scale = np.array([HOST_SCALE[j] for j in range(2, 12)], np.float32)
    for c in range(N_CORES):
        r = res.results[c]["out"]           # (T, P, 10*M) fp16
        np.multiply(
            r.reshape(T, P, NPLANES, M).transpose(0, 1, 3, 2)
             .reshape(S, NPLANES),
            scale, out=full[c * S:(c + 1) * S, 2:])
    return full


# revision 4
# speedup vs baseline: 1.0013x; 1.0013x over previous
"""Legendre polynomials P_0..P_11 (Bonnet recurrence) on 8 TRN2 NeuronCores.

Input:  x float32 [16777216]  (angle cosines in [-1, 1])
Output: float32 [16777216, 12],  out[i, j] = P_j(x[i])

v3: balanced two-engine schedule (v2 post-mortem: GPSIMD tensor_scalar
contends with DVE fp16 tensor_tensor for the shared SBUF port pair --
TT avg inflated 1309 -> 2913 ns -- so GPSIMD is unusable while the DVE
chain runs; v2 hit 242us vs v1's 192us).

  * DVE (19.8us/tile): the 8 TTs (mul3, mul5, 3 pair-chain steps of
    [P,2M] mul+sub) plus the 4 cheapest fp16 4x tensor_scalar ops
    (r, pl4, v5, s9).
  * ACT (17.9us/tile): y=x^2, q4, q5, pl2, s4..s8 (9 ACTIVATE).
  * GPSIMD: nothing.
  * plane2 is stored AS the k=4 chain subtrahend C2*(1.5y-0.5); r and
    s9 are decoded from it by fp16 TS (affine of an affine).
  * x16 comes from the host (pure dtype cast, like the output decode);
    DMA 52.5 MB/core stays under the compute walls.
  * planes live in ONE contiguous [P, 10*M] fp16 tile; 3 output
    dma_starts/tile ([0:4M) planes 2-5, [4M:8M) 6-9, [8M:10M) 10-11)
    keep the store stream smooth.

Host: P0/P1 fill + per-plane fp16 scale decode (pure format transform).
Host-simulated worst-case abs error of this pipeline: 6.0e-3 (gate 2e-2).
"""

import numpy as np

import concourse.bass as bass
import concourse.tile as tile
from concourse import bacc, mybir
from concourse.bass_utils import run_bass_kernel_spmd

N = 16777216
N_CORES = 8
S = N // N_CORES      # 2097152 elements per core
P = 128               # SBUF partitions
M = 2048              # free-dim elements per plane per tile
T = S // (P * M)      # 8 tiles per core
NORD = 12
NPLANES = 10          # device-computed orders 2..11

F32 = mybir.dt.float32
F16 = mybir.dt.float16


def _chain_coef():
    def a(k):
        return (2 * k + 1) / (k + 1)

    def b(k):
        return k / (k + 1)

    coef = {}
    for k in range(4, 10):
        A = a(k + 1) * a(k)
        B = -(b(k + 1) + a(k + 1) * b(k) / a(k - 1))
        G = a(k + 1) * b(k) * b(k - 1) / a(k - 1)
        coef[k] = (A, B, G)
    return coef


CHAIN = _chain_coef()
G4, G5, G6, G7, G8, G9 = (CHAIN[k][2] for k in range(4, 10))

# plane scales: plane_j = C_j * P_j  (host multiplies by 1/C_j)
C3, C5, C6 = 1.0, 1.0, 1.0
C7, C9 = C3 / G5, C5 / G7
C8, C10, C11 = 1.0 / G6, 1.0 / G8, C7 / G9
PHAT4 = C8 * G6            # plane4 = PHAT4 * P4  (the k=6 subtrahend)
C2 = G4 * C6               # plane2 = C2*(1.5y-0.5) (the k=4 subtrahend)

FK = {4: C6 / PHAT4, 5: C7 / C5, 6: C8 / C6, 7: C9 / C7,
      8: C10 / C8, 9: C11 / C9}

# host decode: P_j = HOST_SCALE[j] * plane_j  (pure scales, no shifts)
HOST_SCALE = {2: 1.0 / C2, 3: 1.0 / C3, 4: 1.0 / PHAT4, 5: 1.0 / C5,
              6: 1.0 / C6, 7: 1.0 / C7, 8: 1.0 / C8, 9: 1.0 / C9,
              10: 1.0 / C10, 11: 1.0 / C11}

_NC_CACHE = {}


def build_nc():
    if "nc" in _NC_CACHE:
        return _NC_CACHE["nc"]
    nc = bacc.Bacc("TRN2", target_bir_lowering=False, debug=False,
                   num_devices=N_CORES)
    x = nc.declare_dram_parameter("x", [T, P, M], F32, isOutput=False)
    xh = nc.declare_dram_parameter("x16", [T, P, M], F16, isOutput=False)
    out = nc.declare_dram_parameter("out", [T, P, NPLANES * M], F16,
                                    isOutput=True)

    ACT = mybir.ActivationFunctionType
    ALU = mybir.AluOpType

    with tile.TileContext(nc) as tc:
        with (
            tc.tile_pool(name="cbias", bufs=1) as cpool,
            tc.tile_pool(name="xin", bufs=3) as xpool,
            tc.tile_pool(name="x16", bufs=3) as hpool,
            tc.tile_pool(name="ysq", bufs=2) as ypool,
            tc.tile_pool(name="mega", bufs=2) as mpool,
            tc.tile_pool(name="schain", bufs=6) as spool,
            tc.tile_pool(name="aff", bufs=6) as affpool,
        ):
            b47 = cpool.tile([P, 1], F32)
            nc.vector.memset(b47[:], -3.0 / 7.0)
            b59 = cpool.tile([P, 1], F32)
            nc.vector.memset(b59[:], -5.0 / 9.0)

            xts = {}

            def load_x(t):
                xts[t] = (xpool.tile([P, M], F32, tag="xt", name=f"xt_{t}"),
                          hpool.tile([P, M], F16, tag="xh", name=f"xh_{t}"))
                nc.sync.dma_start(xts[t][0][:], x[t])
                nc.sync.dma_start(xts[t][1][:], xh[t])

            load_x(0)
            load_x(1)
            for t in range(T):
                if t + 2 < T:
                    load_x(t + 2)
                xt, x16 = xts.pop(t)

                mega = mpool.tile([P, NPLANES * M], F16, tag="mega",
                                  name=f"mega_{t}")
                pl = {j: mega[:, (j - 2) * M:(j - 1) * M]
                      for j in range(2, 12)}

                yt = ypool.tile([P, M], F32, tag="yt", name=f"yt_{t}")
                nc.scalar.activation(yt[:], xt[:], ACT.Square)

                # plane2 = C2*(1.5y - 0.5)  == the k=4 chain subtrahend (ACT)
                nc.scalar.activation(pl[2], yt[:], ACT.Copy,
                                     bias=-0.5 * C2, scale=1.5 * C2)

                # r = 2.5y - 1.5 recovered from plane2 (DVE 4x TS)
                r = affpool.tile([P, M], F16, tag="aff", name=f"r_{t}")
                nc.vector.tensor_scalar(r[:], pl[2], 5.0 / (3.0 * C2),
                                        -2.0 / 3.0, ALU.mult, ALU.add)
                # plane3 = x16 * r   (DVE TT)
                nc.vector.tensor_mul(pl[3], x16[:], r[:])

                # plane4 = PHAT4*(4.375 q4 - 3/7),  q4 = (y - 3/7)^2
                q4 = affpool.tile([P, M], F16, tag="aff", name=f"q4_{t}")
                nc.scalar.activation(q4[:], yt[:], ACT.Square, bias=b47[:])
                nc.vector.tensor_scalar(pl[4], q4[:], 4.375 * PHAT4,
                                        -(3.0 / 7.0) * PHAT4,
                                        ALU.mult, ALU.add)

                # plane5 = x16 * (7.875 q5 - 5/9),  q5 = (y - 5/9)^2
                q5 = affpool.tile([P, M], F16, tag="aff", name=f"q5_{t}")
                nc.scalar.activation(q5[:], yt[:], ACT.Square, bias=b59[:])
                v5 = affpool.tile([P, M], F16, tag="aff", name=f"v5_{t}")
                nc.vector.tensor_scalar(v5[:], q5[:], 7.875, -5.0 / 9.0,
                                        ALU.mult, ALU.add)
                nc.vector.tensor_mul(pl[5], x16[:], v5[:])

                # s-pair tiles: [s_k | s_{k+1}], s_k = (A_k y + B_k) * fk
                # s4..s8 on ACT from f32 y; s9 on DVE 4x TS from plane2.
                spairs = {}
                for k in (4, 6, 8):
                    sp = spool.tile([P, 2 * M], F16, tag="s",
                                    name=f"s{k}{k + 1}_{t}")
                    for kk, half in ((k, sp[:, 0:M]), (k + 1, sp[:, M:2 * M])):
                        A, B, _G = CHAIN[kk]
                        if kk == 9:
                            sc = A * FK[kk] / (1.5 * C2)
                            bi = A * FK[kk] / 3.0 + B * FK[kk]
                            nc.vector.tensor_scalar(half, pl[2], sc, bi,
                                                    ALU.mult, ALU.add)
                        else:
                            nc.scalar.activation(half, yt[:], ACT.Copy,
                                                 bias=B * FK[kk],
                                                 scale=A * FK[kk])
                    spairs[k] = sp

                # planes 2,3 are complete before the chain starts
                nc.sync.dma_start(out[t][:, 0:2 * M], mega[:, 0:2 * M])

                # pair chain: planes (k+2,k+3) = s_pair .* (k,k+1) - (k-2,k-1)
                for k in (4, 6, 8):
                    src = mega[:, (k - 2) * M:k * M]
                    sub = mega[:, (k - 4) * M:(k - 2) * M]
                    dst = mega[:, k * M:(k + 2) * M]
                    nc.vector.tensor_mul(dst, spairs[k][:], src)
                    nc.vector.tensor_sub(dst, dst, sub)
                    if k == 4:
                        nc.sync.dma_start(out[t][:, 2 * M:4 * M],
                                          mega[:, 2 * M:4 * M])
                    elif k == 6:
                        nc.sync.dma_start(out[t][:, 4 * M:8 * M],
                                          mega[:, 4 * M:8 * M])
                    else:
                        nc.sync.dma_start(out[t][:, 8 * M:NPLANES * M],
                                          mega[:, 8 * M:NPLANES * M])
    nc.compile()
    _NC_CACHE["nc"] = nc
    return nc


def run_device(x_full, trace=False, **kw):
    nc = build_nc()
    x16_full = x_full.astype(np.float16)
    in_maps = [
        {"x": np.ascontiguousarray(x_full[c * S:(c + 1) * S].reshape(T, P, M)),
         "x16": np.ascontiguousarray(
             x16_full[c * S:(c + 1) * S].reshape(T, P, M))}
        for c in range(N_CORES)
    ]
    return run_bass_kernel_spmd(nc, in_maps, core_ids=list(range(N_CORES)),
                                trace=trace, **kw)


def kernel(x):
    x = np.asarray(x, dtype=np.float32)
    res = run_device(x)
    full = np.empty((N, NORD), np.float32)
    full[:, 0] = 1.0          # P0 == 1 (constant; no compute involved)
    full[:, 1] = x            # P1 == x (identity; no compute involved)
    scale = np.array([HOST_SCALE[j] for j in range(2, 12)], np.float32)
    for c in range(N_CORES):
        r = res.results[c]["out"]           # (T, P, 10*M) fp16
        np.multiply(
            r.reshape(T, P, NPLANES, M).transpose(0, 1, 3, 2)
             .reshape(S, NPLANES),
            scale, out=full[c * S:(c + 1) * S, 2:])
    return full


# revision 5
# speedup vs baseline: 1.0034x; 1.0021x over previous
"""Legendre polynomials P_0..P_11 (Bonnet recurrence) on 8 TRN2 NeuronCores.

Input:  x float32 [16777216]  (angle cosines in [-1, 1])
Output: float32 [16777216, 12],  out[i, j] = P_j(x[i])

v3: balanced two-engine schedule (v2 post-mortem: GPSIMD tensor_scalar
contends with DVE fp16 tensor_tensor for the shared SBUF port pair --
TT avg inflated 1309 -> 2913 ns -- so GPSIMD is unusable while the DVE
chain runs; v2 hit 242us vs v1's 192us).

  * DVE (19.8us/tile): the 8 TTs (mul3, mul5, 3 pair-chain steps of
    [P,2M] mul+sub) plus the 4 cheapest fp16 4x tensor_scalar ops
    (r, pl4, v5, s9).
  * ACT (17.9us/tile): y=x^2, q4, q5, pl2, s4..s8 (9 ACTIVATE).
  * GPSIMD: nothing.
  * plane2 is stored AS the k=4 chain subtrahend C2*(1.5y-0.5); r and
    s9 are decoded from it by fp16 TS (affine of an affine).
  * x16 comes from the host (pure dtype cast, like the output decode);
    DMA 52.5 MB/core stays under the compute walls.
  * planes live in ONE contiguous [P, 10*M] fp16 tile; 3 output
    dma_starts/tile ([0:4M) planes 2-5, [4M:8M) 6-9, [8M:10M) 10-11)
    keep the store stream smooth.

Host: P0/P1 fill + per-plane fp16 scale decode (pure format transform).
Host-simulated worst-case abs error of this pipeline: 6.0e-3 (gate 2e-2).
"""

import numpy as np

import concourse.bass as bass
import concourse.tile as tile
from concourse import bacc, mybir
from concourse.bass_utils import run_bass_kernel_spmd

N = 16777216
N_CORES = 8
S = N // N_CORES      # 2097152 elements per core
P = 128               # SBUF partitions
M = 2048              # free-dim elements per plane per tile
T = S // (P * M)      # 8 tiles per core
NORD = 12
NPLANES = 10          # device-computed orders 2..11

F32 = mybir.dt.float32
F16 = mybir.dt.float16


def _chain_coef():
    def a(k):
        return (2 * k + 1) / (k + 1)

    def b(k):
        return k / (k + 1)

    coef = {}
    for k in range(4, 10):
        A = a(k + 1) * a(k)
        B = -(b(k + 1) + a(k + 1) * b(k) / a(k - 1))
        G = a(k + 1) * b(k) * b(k - 1) / a(k - 1)
        coef[k] = (A, B, G)
    return coef


CHAIN = _chain_coef()
G4, G5, G6, G7, G8, G9 = (CHAIN[k][2] for k in range(4, 10))

# plane scales: plane_j = C_j * P_j  (host multiplies by 1/C_j)
C3, C5, C6 = 1.0, 1.0, 1.0
C7, C9 = C3 / G5, C5 / G7
C8, C10, C11 = 1.0 / G6, 1.0 / G8, C7 / G9
PHAT4 = C8 * G6            # plane4 = PHAT4 * P4  (the k=6 subtrahend)
C2 = G4 * C6               # plane2 = C2*(1.5y-0.5) (the k=4 subtrahend)

FK = {4: C6 / PHAT4, 5: C7 / C5, 6: C8 / C6, 7: C9 / C7,
      8: C10 / C8, 9: C11 / C9}

# host decode: P_j = HOST_SCALE[j] * plane_j  (pure scales, no shifts)
HOST_SCALE = {2: 1.0 / C2, 3: 1.0 / C3, 4: 1.0 / PHAT4, 5: 1.0 / C5,
              6: 1.0 / C6, 7: 1.0 / C7, 8: 1.0 / C8, 9: 1.0 / C9,
              10: 1.0 / C10, 11: 1.0 / C11}

_NC_CACHE = {}


def build_nc():
    if "nc" in _NC_CACHE:
        return _NC_CACHE["nc"]
    nc = bacc.Bacc("TRN2", target_bir_lowering=False, debug=False,
                   num_devices=N_CORES)
    x = nc.declare_dram_parameter("x", [T, P, M], F32, isOutput=False)
    xh = nc.declare_dram_parameter("x16", [T, P, M], F16, isOutput=False)
    out = nc.declare_dram_parameter("out", [T, P, NPLANES * M], F16,
                                    isOutput=True)

    ACT = mybir.ActivationFunctionType
    ALU = mybir.AluOpType

    with tile.TileContext(nc) as tc:
        with (
            tc.tile_pool(name="cbias", bufs=1) as cpool,
            tc.tile_pool(name="xin", bufs=3) as xpool,
            tc.tile_pool(name="x16", bufs=3) as hpool,
            tc.tile_pool(name="ysq", bufs=2) as ypool,
            tc.tile_pool(name="mega", bufs=2) as mpool,
            tc.tile_pool(name="schain", bufs=6) as spool,
            tc.tile_pool(name="aff", bufs=6) as affpool,
        ):
            b47 = cpool.tile([P, 1], F32)
            nc.vector.memset(b47[:], -3.0 / 7.0)
            b59 = cpool.tile([P, 1], F32)
            nc.vector.memset(b59[:], -5.0 / 9.0)

            xts = {}

            def load_x(t):
                xts[t] = (xpool.tile([P, M], F32, tag="xt", name=f"xt_{t}"),
                          hpool.tile([P, M], F16, tag="xh", name=f"xh_{t}"))
                nc.sync.dma_start(xts[t][0][:], x[t])
                nc.sync.dma_start(xts[t][1][:], xh[t])

            load_x(0)
            load_x(1)
            for t in range(T):
                if t + 2 < T:
                    load_x(t + 2)
                xt, x16 = xts.pop(t)

                mega = mpool.tile([P, NPLANES * M], F16, tag="mega",
                                  name=f"mega_{t}")
                pl = {j: mega[:, (j - 2) * M:(j - 1) * M]
                      for j in range(2, 12)}

                yt = ypool.tile([P, M], F32, tag="yt", name=f"yt_{t}")
                nc.scalar.activation(yt[:], xt[:], ACT.Square)

                # plane2 = C2*(1.5y - 0.5)  == the k=4 chain subtrahend (ACT)
                nc.scalar.activation(pl[2], yt[:], ACT.Copy,
                                     bias=-0.5 * C2, scale=1.5 * C2)

                # r = 2.5y - 1.5 recovered from plane2 (DVE 4x TS)
                r = affpool.tile([P, M], F16, tag="aff", name=f"r_{t}")
                nc.vector.tensor_scalar(r[:], pl[2], 5.0 / (3.0 * C2),
                                        -2.0 / 3.0, ALU.mult, ALU.add)
                # plane3 = x16 * r   (DVE TT)
                nc.vector.tensor_mul(pl[3], x16[:], r[:])

                # plane4 = PHAT4*(4.375 q4 - 3/7),  q4 = (y - 3/7)^2
                q4 = affpool.tile([P, M], F16, tag="aff", name=f"q4_{t}")
                nc.scalar.activation(q4[:], yt[:], ACT.Square, bias=b47[:])
                nc.vector.tensor_scalar(pl[4], q4[:], 4.375 * PHAT4,
                                        -(3.0 / 7.0) * PHAT4,
                                        ALU.mult, ALU.add)

                # plane5 = x16 * (7.875 q5 - 5/9),  q5 = (y - 5/9)^2
                q5 = affpool.tile([P, M], F16, tag="aff", name=f"q5_{t}")
                nc.scalar.activation(q5[:], yt[:], ACT.Square, bias=b59[:])
                v5 = affpool.tile([P, M], F16, tag="aff", name=f"v5_{t}")
                nc.vector.tensor_scalar(v5[:], q5[:], 7.875, -5.0 / 9.0,
                                        ALU.mult, ALU.add)
                nc.vector.tensor_mul(pl[5], x16[:], v5[:])

                # s-pair tiles: [s_k | s_{k+1}], s_k = (A_k y + B_k) * fk
                # s4..s8 on ACT from f32 y; s9 on DVE 4x TS from plane2.
                spairs = {}
                for k in (4, 6, 8):
                    sp = spool.tile([P, 2 * M], F16, tag="s",
                                    name=f"s{k}{k + 1}_{t}")
                    for kk, half in ((k, sp[:, 0:M]), (k + 1, sp[:, M:2 * M])):
                        A, B, _G = CHAIN[kk]
                        if kk == 9:
                            sc = A * FK[kk] / (1.5 * C2)
                            bi = A * FK[kk] / 3.0 + B * FK[kk]
                            nc.vector.tensor_scalar(half, pl[2], sc, bi,
                                                    ALU.mult, ALU.add)
                        else:
                            nc.scalar.activation(half, yt[:], ACT.Copy,
                                                 bias=B * FK[kk],
                                                 scale=A * FK[kk])
                    spairs[k] = sp

                # planes 2,3 are complete before the chain starts
                nc.sync.dma_start(out[t][:, 0:2 * M], mega[:, 0:2 * M])

                # pair chain: planes (k+2,k+3) = s_pair .* (k,k+1) - (k-2,k-1)
                for k in (4, 6, 8):
                    src = mega[:, (k - 2) * M:k * M]
                    sub = mega[:, (k - 4) * M:(k - 2) * M]
                    dst = mega[:, k * M:(k + 2) * M]
                    nc.vector.tensor_mul(dst, spairs[k][:], src)
                    nc.vector.tensor_sub(dst, dst, sub)
                    if k == 4:
                        nc.sync.dma_start(out[t][:, 2 * M:4 * M],
                                          mega[:, 2 * M:4 * M])
                        nc.sync.dma_start(out[t][:, 4 * M:6 * M],
                                          mega[:, 4 * M:6 * M])
                    elif k == 6:
                        nc.sync.dma_start(out[t][:, 6 * M:8 * M],
                                          mega[:, 6 * M:8 * M])
                    else:
                        nc.sync.dma_start(out[t][:, 8 * M:NPLANES * M],
                                          mega[:, 8 * M:NPLANES * M])
    nc.compile()
    _NC_CACHE["nc"] = nc
    return nc


def run_device(x_full, trace=False, **kw):
    nc = build_nc()
    x16_full = x_full.astype(np.float16)
    in_maps = [
        {"x": np.ascontiguousarray(x_full[c * S:(c + 1) * S].reshape(T, P, M)),
         "x16": np.ascontiguousarray(
             x16_full[c * S:(c + 1) * S].reshape(T, P, M))}
        for c in range(N_CORES)
    ]
    return run_bass_kernel_spmd(nc, in_maps, core_ids=list(range(N_CORES)),
                                trace=trace, **kw)


def kernel(x):
    x = np.asarray(x, dtype=np.float32)
    res = run_device(x)
    full = np.empty((N, NORD), np.float32)
    full[:, 0] = 1.0          # P0 == 1 (constant; no compute involved)
    full[:, 1] = x            # P1 == x (identity; no compute involved)
    scale = np.array([HOST_SCALE[j] for j in range(2, 12)], np.float32)
    for c in range(N_CORES):
        r = res.results[c]["out"]           # (T, P, 10*M) fp16
        np.multiply(
            r.reshape(T, P, NPLANES, M).transpose(0, 1, 3, 2)
             .reshape(S, NPLANES),
            scale, out=full[c * S:(c + 1) * S, 2:])
    return full
